# revision 1
# baseline (speedup 1.0000x reference)
"""Trainium2 Bass kernel for nn_EncoderBlock (dense transformer encoder block).

Strategy (8 NeuronCores):
  - Tokens sharded 512/core (cores 0-3: batch 0, cores 4-7: batch 1).
  - Activations kept feature-major ("transposed", [feature, token]) on chip so
    every matmul is lhsT=W (K-major), rhs=X^T with zero on-chip transposes.
  - Per core: LN1 -> K^T,V,Q^T projections for its tokens (all heads).
    AllGather of K^T and V within each 4-core batch group -> full-batch K/V.
    Attention for its 512 query tokens, all 16 heads (scores computed
    transposed [k,q]; softmax sum folded into the AV matmul via a ones
    column appended to V; no max-subtraction needed at these magnitudes).
    W_o + residual, LN2, FFN (full weights, token-parallel), residual.
  - Matmuls in float32r (TF32): 1 cycle/row on the PE vs 4 for fp32.
  - Host: shards/transposes inputs, TF32-rounds weights, gathers output.
"""

import os

import numpy as np

import concourse.bass as bass
import concourse.mybir as mybir
import concourse.tile as tile
from concourse import bacc
from concourse.bass_utils import run_bass_kernel_spmd

N_CORES = 8
GRP = 4          # cores per batch group
P = 128
TOK = 512        # tokens per core
S = 2048         # sequence length (tokens per batch)
D = 1024
KT = D // P      # 8 feature tiles
H = 16
DK = 64
DK1 = DK + 1
F = 4096
FT = F // P      # 32 ffn tiles
EPS = 1e-6
SCALE = 0.125    # 1/sqrt(DK)
MT_S = S // P    # 16 k-token tiles per batch
NBLK = 3         # score blocks per psum tile / exp call

f32 = mybir.dt.float32
f32r = mybir.dt.float32r
ALU = mybir.AluOpType
ACT = mybir.ActivationFunctionType


def tf32_round(x: np.ndarray) -> np.ndarray:
    u = np.ascontiguousarray(x, dtype=np.float32).view(np.uint32)
    lsb = (u >> np.uint32(13)) & np.uint32(1)
    u = u + np.uint32(0x0FFF) + lsb
    u = u & np.uint32(0xFFFFE000)
    return u.view(np.float32)


def _layer_norm(nc, tc, hpool, ones_r, x_tiles, g_t, b_t, tag):
    """Feature-major layernorm over KT [128, TOK] f32 tiles.

    Per-token (free-dim) stats via ones-matmul partition reduction on PE.
    Returns KT f32r tiles allocated from hpool.
    """
    rows = tc.alloc_tile_pool(name=f"lnrow_{tag}", bufs=1)
    ltr = tc.alloc_tile_pool(name=f"lntr_{tag}", bufs=2)
    lnps = tc.alloc_tile_pool(name=f"lnps_{tag}", bufs=1, space="PSUM")

    ps_sum = lnps.tile([1, TOK], f32, name=f"pssum_{tag}", tag="pssum")
    ps_sq = lnps.tile([1, TOK], f32, name=f"pssq_{tag}", tag="pssq")
    for kt in range(KT):
        xr = ltr.tile([P, TOK], f32r, name=f"xr_{tag}_{kt}", tag="xrsq", bufs=2)
        sq = ltr.tile([P, TOK], f32r, name=f"sq_{tag}_{kt}", tag="xrsq", bufs=2)
        nc.vector.tensor_copy(xr[:], x_tiles[kt][:])
        nc.vector.tensor_mul(sq[:], x_tiles[kt][:], x_tiles[kt][:])
        nc.tensor.matmul(ps_sum[:], lhsT=ones_r[:], rhs=xr[:],
                         start=(kt == 0), stop=(kt == KT - 1))
        nc.tensor.matmul(ps_sq[:], lhsT=ones_r[:], rhs=sq[:],
                         start=(kt == 0), stop=(kt == KT - 1))

    inv_n = 1.0 / D
    mean = rows.tile([1, TOK], f32, name=f"mean_{tag}", tag="mean")
    var = rows.tile([1, TOK], f32, name=f"var_{tag}", tag="var")
    msq = rows.tile([1, TOK], f32, name=f"msq_{tag}", tag="msqlnv")
    nc.vector.tensor_scalar_mul(mean[:], ps_sum[:], inv_n)
    nc.vector.tensor_scalar_mul(var[:], ps_sq[:], inv_n)
    nc.vector.tensor_mul(msq[:], mean[:], mean[:])
    nc.vector.tensor_sub(var[:], var[:], msq[:])
    nc.vector.tensor_scalar_add(var[:], var[:], EPS)

    # r = 1/sqrt(var+eps) = exp(-0.5*ln(var+eps))
    lnv = rows.tile([1, TOK], f32, name=f"lnv_{tag}", tag="msqlnv")
    nc.scalar.activation(lnv[:], var[:], ACT.Ln)
    r_row = rows.tile([1, TOK], f32, name=f"r_{tag}", tag="r")
    nc.scalar.activation(r_row[:], lnv[:], ACT.Exp, scale=-0.5)
    mr_row = rows.tile([1, TOK], f32, name=f"mr_{tag}", tag="mr")
    nc.vector.tensor_mul(mr_row[:], mean[:], r_row[:])

    r_bc = rows.tile([P, TOK], f32, name=f"rbc_{tag}", tag="rbc")
    mr_bc = rows.tile([P, TOK], f32, name=f"mrbc_{tag}", tag="mrbc")
    nc.gpsimd.partition_broadcast(r_bc[:], r_row[:])
    nc.gpsimd.partition_broadcast(mr_bc[:], mr_row[:])

    h_tiles = []
    for kt in range(KT):
        t1 = ltr.tile([P, TOK], f32, name=f"t1_{tag}_{kt}", tag="lnt1", bufs=1)
        h = hpool.tile([P, TOK], f32r, name=f"h_{tag}_{kt}", tag=f"h_{kt}")
        nc.vector.tensor_mul(t1[:], x_tiles[kt][:], r_bc[:])
        nc.vector.tensor_sub(t1[:], t1[:], mr_bc[:])
        nc.vector.tensor_scalar(h[:], t1[:], g_t[:, kt:kt + 1],
                                b_t[:, kt:kt + 1], ALU.mult, ALU.add)
        h_tiles.append(h)
    lnps.release()
    ltr.release()
    rows.release()
    return h_tiles


def build(n_iters: int = 1):
    nc = bacc.Bacc("TRN2", target_bir_lowering=False, debug=False,
                   num_devices=N_CORES)

    xT = nc.dram_tensor("xT", [D, TOK], f32, kind="ExternalInput").ap()
    wq = nc.dram_tensor("wq", [D, D], f32r, kind="ExternalInput").ap()
    wk = nc.dram_tensor("wk", [D, D], f32r, kind="ExternalInput").ap()
    wv = nc.dram_tensor("wv", [D, D], f32r, kind="ExternalInput").ap()
    wo = nc.dram_tensor("wo", [D, D], f32r, kind="ExternalInput").ap()
    w1 = nc.dram_tensor("w1", [D, F], f32r, kind="ExternalInput").ap()
    w2 = nc.dram_tensor("w2", [F, D], f32r, kind="ExternalInput").ap()
    bo_v = nc.dram_tensor("bo_v", [P, KT], f32, kind="ExternalInput").ap()
    b1_v = nc.dram_tensor("b1_v", [P, FT], f32, kind="ExternalInput").ap()
    b2_v = nc.dram_tensor("b2_v", [P, KT], f32, kind="ExternalInput").ap()
    g1_v = nc.dram_tensor("g1_v", [P, KT], f32, kind="ExternalInput").ap()
    be1_v = nc.dram_tensor("be1_v", [P, KT], f32, kind="ExternalInput").ap()
    g2_v = nc.dram_tensor("g2_v", [P, KT], f32, kind="ExternalInput").ap()
    be2_v = nc.dram_tensor("be2_v", [P, KT], f32, kind="ExternalInput").ap()

    outT = nc.dram_tensor("outT", [D, TOK], f32, kind="ExternalOutput").ap()

    groups = [[0, 1, 2, 3], [4, 5, 6, 7]]

    with tile.TileContext(nc) as tc:
        sb = tc.alloc_tile_pool(name="sb", bufs=1)
        tr = tc.alloc_tile_pool(name="tr", bufs=3)
        wp = tc.alloc_tile_pool(name="wp", bufs=11)
        ep = tc.alloc_tile_pool(name="ep", bufs=2)
        dram = tc.alloc_tile_pool(name="dram", bufs=1, space="DRAM")
        xp = tc.alloc_tile_pool(name="xp", bufs=1)

        # ---- constants / small inputs ----
        ones_f = sb.tile([P, 32], f32, name="ones_f", tag="ones_f")
        nc.vector.memset(ones_f[:], 1.0)
        ones_r = sb.tile([P, 1], f32r, name="ones_r", tag="ones_r")
        nc.vector.tensor_copy(ones_r[:], ones_f[:, 0:1])
        bo_t = sb.tile([P, KT], f32, name="bo_t", tag="bo_t")
        b1_t = sb.tile([P, FT], f32, name="b1_t", tag="b1_t")
        b2_t = sb.tile([P, KT], f32, name="b2_t", tag="b2_t")
        g1_t = sb.tile([P, KT], f32, name="g1_t", tag="g1_t")
        be1_t = sb.tile([P, KT], f32, name="be1_t", tag="be1_t")
        g2_t = sb.tile([P, KT], f32, name="g2_t", tag="g2_t")
        be2_t = sb.tile([P, KT], f32, name="be2_t", tag="be2_t")
        for t, src in [(bo_t, bo_v), (b1_t, b1_v), (b2_t, b2_v),
                       (g1_t, g1_v), (be1_t, be1_v), (g2_t, g2_v),
                       (be2_t, be2_v)]:
            nc.sync.dma_start(out=t[:], in_=src)

        # ---- load x^T shard ----
        x_tiles = []
        for kt in range(KT):
            xt = xp.tile([P, TOK], f32, name=f"x_{kt}", tag=f"x_{kt}")
            nc.sync.dma_start(out=xt[:], in_=xT[kt * P:(kt + 1) * P, :])
            x_tiles.append(xt)

        kT_sh = dram.tile([D, TOK], f32r, name="kT_sh", tag="kT_sh")
        kT_full = dram.tile([GRP * D, TOK], f32r, name="kT_full", tag="kT_full")
        v_sh = dram.tile([TOK, D], f32r, name="v_sh", tag="v_sh")
        v_full = dram.tile([S, D], f32r, name="v_full", tag="v_full")

        for it in range(n_iters):
            # ================= LN1 + QKV =================
            x2p = tc.alloc_tile_pool(name=f"x2p{it}", bufs=1)
            cp = tc.alloc_tile_pool(name=f"cp{it}", bufs=1)
            qp = tc.alloc_tile_pool(name=f"qp{it}", bufs=1)
            hp = tc.alloc_tile_pool(name=f"hp{it}", bufs=1)
            h_tiles = _layer_norm(nc, tc, hp, ones_r, x_tiles, g1_t, be1_t, f"ln1_{it}")

            qkvps = tc.alloc_tile_pool(name=f"qkvps{it}", bufs=4, space="PSUM")
            # K^T projection first (feeds AllGather)
            wk_tiles = []
            for kt in range(KT):
                wkt = wp.tile([P, D], f32r, name=f"wk_{kt}", tag="w")
                nc.sync.dma_start(out=wkt[:], in_=wk[kt * P:(kt + 1) * P, :])
                wk_tiles.append(wkt)
            for mt in range(KT):
                ps = qkvps.tile([P, TOK], f32, name=f"psk_{mt}", tag="qkv")
                for kt in range(KT):
                    nc.tensor.matmul(ps[:], lhsT=wk_tiles[kt][:, mt * P:(mt + 1) * P],
                                     rhs=h_tiles[kt][:], start=(kt == 0),
                                     stop=(kt == KT - 1))
                kev = tr.tile([P, TOK], f32r, name=f"kev_{mt}", tag="kev", bufs=2)
                nc.vector.tensor_copy(kev[:], ps[:])
                nc.sync.dma_start(out=kT_sh[mt * P:(mt + 1) * P, :], in_=kev[:])
            nc.gpsimd.collective_compute(
                "AllGather", ALU.bypass, ins=[kT_sh[:].opt()],
                outs=[kT_full[:].opt()], replica_groups=groups)

            # V projection (token-major) + AllGather
            wv_tiles = []
            for kt in range(KT):
                wvt = wp.tile([P, D], f32r, name=f"wv_{kt}", tag="w")
                nc.sync.dma_start(out=wvt[:], in_=wv[kt * P:(kt + 1) * P, :])
                wv_tiles.append(wvt)
            for mt in range(TOK // P):
                for nt in range(2):
                    ps = qkvps.tile([P, TOK], f32, name=f"psv_{mt}_{nt}", tag="qkv")
                    for kt in range(KT):
                        nc.tensor.matmul(
                            ps[:], lhsT=h_tiles[kt][:, mt * P:(mt + 1) * P],
                            rhs=wv_tiles[kt][:, nt * TOK:(nt + 1) * TOK],
                            start=(kt == 0), stop=(kt == KT - 1))
                    vev = tr.tile([P, TOK], f32r, name=f"vev_{mt}_{nt}", tag="vev", bufs=2)
                    nc.vector.tensor_copy(vev[:], ps[:])
                    nc.sync.dma_start(
                        out=v_sh[mt * P:(mt + 1) * P, nt * TOK:(nt + 1) * TOK],
                        in_=vev[:])
            nc.gpsimd.collective_compute(
                "AllGather", ALU.bypass, ins=[v_sh[:].opt()],
                outs=[v_full[:].opt()], replica_groups=groups)

            # Q^T projection (resident)
            wq_tiles = []
            for kt in range(KT):
                wqt = wp.tile([P, D], f32r, name=f"wq_{kt}", tag="w")
                nc.sync.dma_start(out=wqt[:], in_=wq[kt * P:(kt + 1) * P, :])
                wq_tiles.append(wqt)
            q_tiles = []
            for mt in range(KT):
                ps = qkvps.tile([P, TOK], f32, name=f"psq_{mt}", tag="qkv")
                for kt in range(KT):
                    nc.tensor.matmul(ps[:], lhsT=wq_tiles[kt][:, mt * P:(mt + 1) * P],
                                     rhs=h_tiles[kt][:], start=(kt == 0),
                                     stop=(kt == KT - 1))
                qt = qp.tile([P, TOK], f32r, name=f"q_{mt}", tag=f"q_{mt}")
                nc.vector.tensor_copy(qt[:], ps[:])
                q_tiles.append(qt)
            qkvps.release()
            hp.release()

            # ================= attention =================
            wo_tiles = []
            for kt in range(KT):
                wot = wp.tile([P, D], f32r, name=f"wo_{kt}", tag="w")
                nc.sync.dma_start(out=wot[:], in_=wo[kt * P:(kt + 1) * P, :])
                wo_tiles.append(wot)
            scps = tc.alloc_tile_pool(name=f"scps{it}", bufs=2, space="PSUM")
            ctxps = tc.alloc_tile_pool(name=f"ctxps{it}", bufs=2, space="PSUM")
            ctx_tiles = []
            for p in range(KT):
                ct = cp.tile([P, TOK], f32r, name=f"ctx_{p}", tag=f"ctx_{p}")
                ctx_tiles.append(ct)
            for p in range(KT):
                ktp = tr.tile([P, S], f32r, name=f"ktp_{p}", tag="ktp", bufs=1)
                nc.sync.dma_start(
                    out=ktp[:].rearrange("d (c t) -> d c t", c=GRP),
                    in_=kT_full[:].rearrange("(c d) t -> d c t", d=D)
                    [p * P:(p + 1) * P, :, :])
                # V columns for this pair, [V_a|1|V_b|1] per k-token tile
                vp = tr.tile([P, MT_S * 2 * DK1], f32r, name=f"vp_{p}", tag="vp",
                             bufs=2)
                for hh in range(2):
                    nc.sync.dma_start(
                        out=vp[:].rearrange("q (m x) -> q m x", x=2 * DK1)
                        [:, :, hh * DK1:hh * DK1 + DK],
                        in_=v_full[:, p * P + hh * DK:p * P + (hh + 1) * DK]
                        .rearrange("(m q) d -> q m d", q=P))
                nc.vector.tensor_copy(
                    vp[:].rearrange("q (k c) -> q k c", c=DK1)[:, :, DK:DK1]
                    .squeeze(2), ones_f[:])

                ps_ctx = [ctxps.tile([DK1, TOK], f32, name=f"psctx_{p}_{hh}",
                                     tag="psctx") for hh in range(2)]
                ps_sc = None
                e_t = None
                nb = 0
                for i in range(2 * MT_S):
                    mt, half = i >> 1, i & 1
                    j = i % NBLK
                    if j == 0:
                        nb = min(NBLK, 2 * MT_S - i)
                        ps_sc = scps.tile([P, NBLK * TOK], f32, name=f"pssc_{p}_{i}",
                                          tag="pssc")
                        e_t = ep.tile([P, NBLK * TOK], f32r, name=f"e_{p}_{i}",
                                      tag="e")
                    nc.tensor.matmul(
                        ps_sc[:, j * TOK:(j + 1) * TOK],
                        lhsT=ktp[half * DK:(half + 1) * DK, mt * P:(mt + 1) * P],
                        rhs=q_tiles[p][half * DK:(half + 1) * DK, :],
                        start=True, stop=True)
                    if j == nb - 1:
                        nc.scalar.activation(e_t[:, 0:nb * TOK], ps_sc[:, 0:nb * TOK],
                                             ACT.Exp, scale=SCALE)
                        for jj in range(nb):
                            ii = i - nb + 1 + jj
                            mtt, hf = ii >> 1, ii & 1
                            nc.tensor.matmul(
                                ps_ctx[hf][:],
                                lhsT=vp[:].rearrange("q (m c) -> q m c", c=2 * DK1)
                                [:, mtt, hf * DK1:(hf + 1) * DK1],
                                rhs=e_t[:, jj * TOK:(jj + 1) * TOK],
                                start=(mtt == 0), stop=(mtt == MT_S - 1))
                # normalize: ctx = ctx_unnorm / sumexp
                for hf in range(2):
                    rec = tr.tile([1, TOK], f32, name=f"rec_{p}_{hf}", tag="rec", bufs=2)
                    nc.vector.reciprocal(rec[:], ps_ctx[hf][DK:DK1, :])
                    rbc = tr.tile([DK, TOK], f32, name=f"rbc_{p}_{hf}", tag="recbc", bufs=2)
                    nc.gpsimd.partition_broadcast(rbc[:], rec[:])
                    if hf == 0:
                        nc.vector.tensor_mul(ctx_tiles[p][0:DK, :],
                                             ps_ctx[hf][0:DK, :], rbc[:])
                    else:
                        shift = tr.tile([DK, TOK], f32r, name=f"sh_{p}", tag="shift", bufs=2)
                        nc.vector.tensor_mul(shift[:], ps_ctx[hf][0:DK, :], rbc[:])
                        nc.sync.dma_start(out=ctx_tiles[p][DK:P, :], in_=shift[:])
            ctxps.release()
            scps.release()
            qp.release()

            # ================= W_o + residual -> x2 =================
            wops = tc.alloc_tile_pool(name=f"wops{it}", bufs=4, space="PSUM")
            x2_tiles = []
            for mt in range(KT):
                ps = wops.tile([P, TOK], f32, name=f"pso_{mt}", tag="wo")
                for kt in range(KT):
                    nc.tensor.matmul(ps[:], lhsT=wo_tiles[kt][:, mt * P:(mt + 1) * P],
                                     rhs=ctx_tiles[kt][:], start=(kt == 0),
                                     stop=(kt == KT - 1))
                x2 = x2p.tile([P, TOK], f32, name=f"x2_{mt}", tag=f"x2_{mt}")
                nc.vector.scalar_tensor_tensor(x2[:], ps[:], bo_t[:, mt:mt + 1],
                                               x_tiles[mt][:], ALU.add, ALU.add)
                x2_tiles.append(x2)
            wops.release()
            cp.release()

            # ================= LN2 + FFN =================
            h2p = tc.alloc_tile_pool(name=f"h2p{it}", bufs=1)
            h2_tiles = _layer_norm(nc, tc, h2p, ones_r, x2_tiles, g2_t, be2_t, f"ln2_{it}")

            apool = tc.alloc_tile_pool(name=f"ap{it}", bufs=8)
            f1ps = tc.alloc_tile_pool(name=f"f1ps{it}", bufs=4, space="PSUM")
            f2ps = tc.alloc_tile_pool(name=f"f2ps{it}", bufs=4, space="PSUM")
            for mg in range(4):
                w1_tiles = []
                for kt in range(KT):
                    w1t = wp.tile([P, D], f32r, name=f"w1_{mg}_{kt}", tag="w")
                    nc.sync.dma_start(
                        out=w1t[:], in_=w1[kt * P:(kt + 1) * P, mg * D:(mg + 1) * D])
                    w1_tiles.append(w1t)
                a_tiles = []
                for mt in range(KT):
                    m = mg * KT + mt
                    ps = f1ps.tile([P, TOK], f32, name=f"psf1_{m}", tag="f1")
                    for kt in range(KT):
                        nc.tensor.matmul(
                            ps[:], lhsT=w1_tiles[kt][:, mt * P:(mt + 1) * P],
                            rhs=h2_tiles[kt][:], start=(kt == 0), stop=(kt == KT - 1))
                    at = apool.tile([P, TOK], f32r, name=f"a_{m}", tag="a")
                    nc.vector.tensor_scalar(at[:], ps[:], b1_t[:, m:m + 1], 0.0,
                                            ALU.add, ALU.max)
                    a_tiles.append(at)
                # FFN2 partial for this mg, accumulate into x2
                w2_tiles = []
                for kt in range(KT):
                    kt2 = mg * KT + kt
                    w2t = wp.tile([P, D], f32r, name=f"w2_{kt2}", tag="w")
                    nc.sync.dma_start(out=w2t[:], in_=w2[kt2 * P:(kt2 + 1) * P, :])
                    w2_tiles.append(w2t)
                for mt in range(KT):
                    ps2 = f2ps.tile([P, TOK], f32, name=f"psf2_{mg}_{mt}", tag="f2")
                    for kt in range(KT):
                        nc.tensor.matmul(
                            ps2[:], lhsT=w2_tiles[kt][:, mt * P:(mt + 1) * P],
                            rhs=a_tiles[kt][:], start=(kt == 0), stop=(kt == KT - 1))
                    nc.vector.tensor_add(x2_tiles[mt][:], x2_tiles[mt][:], ps2[:])
            f2ps.release()
            f1ps.release()
            apool.release()
            h2p.release()

            # ---- final bias + store ----
            for mt in range(KT):
                ot = tr.tile([P, TOK], f32, name=f"ot_{mt}", tag="ot", bufs=2)
                nc.vector.tensor_scalar_add(ot[:], x2_tiles[mt][:], b2_t[:, mt:mt + 1])
                nc.sync.dma_start(out=outT[mt * P:(mt + 1) * P, :], in_=ot[:])

            x2p.release()

        xp.release()
        dram.release()
        ep.release()
        wp.release()
        tr.release()
        sb.release()

    nc.compile()
    return nc


_NC_CACHE = {}


def _get_nc(n_iters: int = 1):
    if n_iters not in _NC_CACHE:
        _NC_CACHE[n_iters] = build(n_iters)
    return _NC_CACHE[n_iters]


def _pack_vec(v: np.ndarray, nt: int) -> np.ndarray:
    return np.ascontiguousarray(v.reshape(nt, P).T, dtype=np.float32)


def run(inputs: dict, trace: bool = False):
    x = np.asarray(inputs["x"], dtype=np.float32)
    B, S_, D_ = x.shape
    assert (B, S_, D_) == (2, S, D)
    xf = x.reshape(B * S_, D_)

    shared = {
        "wq": tf32_round(inputs["W_q"]),
        "wk": tf32_round(inputs["W_k"]),
        "wv": tf32_round(inputs["W_v"]),
        "wo": tf32_round(inputs["W_o"]),
        "w1": tf32_round(inputs["W1"]),
        "w2": tf32_round(inputs["W2"]),
        "bo_v": _pack_vec(np.asarray(inputs["b_o"], np.float32), KT),
        "b1_v": _pack_vec(np.asarray(inputs["b1"], np.float32), FT),
        "b2_v": _pack_vec(np.asarray(inputs["b2"], np.float32), KT),
        "g1_v": _pack_vec(np.asarray(inputs["ln1_g"], np.float32), KT),
        "be1_v": _pack_vec(np.asarray(inputs["ln1_b"], np.float32), KT),
        "g2_v": _pack_vec(np.asarray(inputs["ln2_g"], np.float32), KT),
        "be2_v": _pack_vec(np.asarray(inputs["ln2_b"], np.float32), KT),
    }
    in_maps = []
    for c in range(N_CORES):
        xT_c = np.ascontiguousarray(xf[c * TOK:(c + 1) * TOK, :].T)
        in_maps.append({"xT": xT_c, **shared})

    nc = _get_nc(1)
    res = run_bass_kernel_spmd(nc, in_maps, list(range(N_CORES)), trace=trace)
    out = np.empty((B * S_, D_), dtype=np.float32)
    for c in range(N_CORES):
        out[c * TOK:(c + 1) * TOK, :] = res.results[c]["outT"].T
    return out.reshape(B, S_, D_), res


def kernel(**inputs) -> np.ndarray:
    out, _ = run(inputs, trace=False)
    return out



# revision 41
# speedup vs baseline: 1.8872x; 1.8872x over previous
"""Trainium2 Bass kernel for nn_EncoderBlock (dense transformer encoder block).

Strategy (8 NeuronCores):
  - Tokens sharded 512/core (cores 0-3: batch 0, cores 4-7: batch 1);
    K/V exchanged within each 4-core batch group by ONE AllGather.
  - All matmul operands bf16 (PE streams bf16 at the same 1 col/cycle as
    f32r/tf32; halves DMA traffic, collective bytes, and SBUF footprint,
    and doubles DVE throughput on 16-bit outs). Accumulation stays f32 in
    PSUM; the residual stream and LN statistics stay f32.
  - Weights ship host-pre-tiled ([128, kt*M], kt = contraction tile) so
    every weight load is ONE DMA with contiguous per-partition runs; bulk
    prefetches (W_o, FFN group 0) issue on the scalar engine's HWDGE queue
    to keep the sync queue free for critical-path transfers.
  - Per core: LN1 -> K^T proj -> V proj -> single AllGather of the packed
    [K^T | V-interleaved] buffer (the collective's cost is fixed-overhead
    dominated, so one gather beats two) -> Q^T proj (overlaps the gather)
    -> attention -> W_o + residual (LN2 statistics interleaved into the
    W_o loop) -> LN2 finish -> FFN -> residual.
  - Attention: scores are computed transposed ([k-tokens, q]) so the
    softmax denominator comes free via ones columns embedded in the
    interleaved V layout [va(64)|1|vb(64)|1] (130 cols per head pair):
    AV lhsT hf0 = [va|1], hf1 = [vb|1], accumulated over k-tiles in PSUM;
    exp of raw scores on the ACT engine (no max-subtraction needed at
    these magnitudes); per-half 1/sumexp normalize, with the second
    half's 64 ctx rows moved into place by a small SBUF-to-SBUF DMA.
    The two half-heads' score matmuls land on disjoint PE row groups
    (base partitions 0/64) and pack concurrently in the 128x128 array.
"""

import numpy as np
import ml_dtypes

import concourse.bass as bass
import concourse.mybir as mybir
import concourse.tile as tile
from concourse import bacc
from concourse.bass_utils import run_bass_kernel_spmd

N_CORES = 8
GRP = 4          # cores per batch group
P = 128
TOK = 512        # tokens per core
S = 2048         # sequence length (tokens per batch)
D = 1024
KT = D // P      # 8 feature tiles
H = 16
DK = 64
DK1 = DK + 1
VBLK = 2 * DK1       # 130: [va(64) | 1 | vb(64) | 1]
F = 4096
FT = F // P      # 32 ffn tiles
EPS = 1e-6
SCALE = 0.125    # 1/sqrt(DK)
# DVE bit-trick exp: int16(x*EXP_A + EXP_B) bitcast as bf16 == exp(x*SCALE)
# (Schraudolph in bf16 bit-space; max rel err ~3.3%, washed out by softmax
# normalization). Offloads half the softmax exp from ACT to DVE.
EXP_A = float(np.log2(np.e) * SCALE * 128.0)
EXP_B = 16256.0 - 5.7
MT_S = S // P    # 16 k-token tiles per batch
NBLK = 2         # score blocks per psum tile / exp call

f32 = mybir.dt.float32
bf16 = mybir.dt.bfloat16
ALU = mybir.AluOpType
ACT = mybir.ActivationFunctionType
BF16 = ml_dtypes.bfloat16


def tile_lhsT(w: np.ndarray) -> np.ndarray:
    """[K, M] -> [128, (K//128)*M] bf16; block kt holds rows kt*128..+128."""
    K, M = w.shape
    return np.ascontiguousarray(
        np.asarray(w, np.float32).reshape(K // P, P, M)
        .transpose(1, 0, 2).reshape(P, (K // P) * M).astype(BF16))


def _ln_begin(nc, tc, tag):
    rows = tc.alloc_tile_pool(name=f"lnrow_{tag}", bufs=1)
    ltr = tc.alloc_tile_pool(name=f"lntr_{tag}", bufs=2)
    lnps = tc.alloc_tile_pool(name=f"lnps_{tag}", bufs=1, space="PSUM")
    ps_sum = lnps.tile([1, TOK], f32, name=f"pssum_{tag}", tag="pssum")
    ps_sq = lnps.tile([1, TOK], f32, name=f"pssq_{tag}", tag="pssq")
    return dict(rows=rows, ltr=ltr, lnps=lnps, ps_sum=ps_sum, ps_sq=ps_sq,
                tag=tag)


def _ln_accum(nc, st, ones_r, x_ap, kt, pre_bf16=False):
    tag, ltr = st["tag"], st["ltr"]
    sq = ltr.tile([P, TOK], bf16, name=f"sq_{tag}_{kt}", tag="xrsq", bufs=2)
    nc.vector.tensor_mul(sq[:], x_ap, x_ap)
    if pre_bf16:
        xr_ap = x_ap
    else:
        xr = ltr.tile([P, TOK], bf16, name=f"xr_{tag}_{kt}", tag="xrsq",
                      bufs=2)
        nc.vector.tensor_copy(xr[:], x_ap)
        xr_ap = xr[:]
    nc.tensor.matmul(st["ps_sum"][:], lhsT=ones_r[:], rhs=xr_ap,
                     start=(kt == 0), stop=(kt == KT - 1))
    nc.tensor.matmul(st["ps_sq"][:], lhsT=ones_r[:], rhs=sq[:],
                     start=(kt == 0), stop=(kt == KT - 1))


def _ln_finish(nc, st, hpool, x_tiles, g_t, b_t):
    """Stats row math + normalize -> bf16 h tiles."""
    tag, rows, ltr = st["tag"], st["rows"], st["ltr"]
    ps_sum, ps_sq = st["ps_sum"], st["ps_sq"]
    inv_n = 1.0 / D
    mean = rows.tile([1, TOK], f32, name=f"mean_{tag}", tag="mean")
    var = rows.tile([1, TOK], f32, name=f"var_{tag}", tag="var")
    msq = rows.tile([1, TOK], f32, name=f"msq_{tag}", tag="msqlnv")
    nc.vector.tensor_scalar_mul(mean[:], ps_sum[:], inv_n)
    nc.vector.tensor_scalar_mul(var[:], ps_sq[:], inv_n)
    nc.vector.tensor_mul(msq[:], mean[:], mean[:])
    nc.vector.tensor_sub(var[:], var[:], msq[:])
    nc.vector.tensor_scalar_add(var[:], var[:], EPS)

    # r = 1/sqrt(var+eps) = exp(-0.5*ln(var+eps)); r reuses var's tile,
    # mr (= mean*r) is computed in place over mean.
    lnv = rows.tile([1, TOK], f32, name=f"lnv_{tag}", tag="msqlnv")
    nc.scalar.activation(lnv[:], var[:], ACT.Ln)
    r_row = var
    nc.scalar.activation(r_row[:], lnv[:], ACT.Exp, scale=-0.5)
    mr_row = mean
    nc.vector.tensor_mul(mr_row[:], mean[:], r_row[:])

    r_bc = rows.tile([P, TOK], f32, name=f"rbc_{tag}", tag="rbc")
    mr_bc = rows.tile([P, TOK], f32, name=f"mrbc_{tag}", tag="mrbc")
    nc.gpsimd.partition_broadcast(r_bc[:], r_row[:])
    nc.gpsimd.partition_broadcast(mr_bc[:], mr_row[:])

    h_tiles = []
    for kt in range(KT):
        t1 = ltr.tile([P, TOK], bf16, name=f"t1_{tag}_{kt}", tag="lnt1", bufs=1)
        h = hpool.tile([P, TOK], bf16, name=f"h_{tag}_{kt}", tag=f"h_{kt}")
        nc.vector.tensor_mul(t1[:], x_tiles[kt], r_bc[:])
        nc.vector.tensor_sub(t1[:], t1[:], mr_bc[:])
        nc.vector.tensor_scalar(h[:], t1[:], g_t[:, kt:kt + 1],
                                b_t[:, kt:kt + 1], ALU.mult, ALU.add)
        h_tiles.append(h)
    st["lnps"].release()
    ltr.release()
    rows.release()
    return h_tiles


def _layer_norm(nc, tc, hpool, ones_r, x_tiles, g_t, b_t, tag,
                pre_bf16=False):
    st = _ln_begin(nc, tc, tag)
    for kt in range(KT):
        _ln_accum(nc, st, ones_r, x_tiles[kt], kt, pre_bf16=pre_bf16)
    return _ln_finish(nc, st, hpool, x_tiles, g_t, b_t)


def build(n_iters: int = 1):
    nc = bacc.Bacc("TRN2", target_bir_lowering=False, debug=False,
                   num_devices=N_CORES)

    xT = nc.dram_tensor("xT", [D, TOK], f32, kind="ExternalInput").ap()
    xbf = nc.dram_tensor("xbf", [P, KT * TOK], bf16, kind="ExternalInput").ap()
    wq = nc.dram_tensor("wq", [P, KT * D], bf16, kind="ExternalInput").ap()
    wk = nc.dram_tensor("wk", [P, KT * D], bf16, kind="ExternalInput").ap()
    wv = nc.dram_tensor("wv", [P, KT * D], bf16, kind="ExternalInput").ap()
    wo = nc.dram_tensor("wo", [P, KT * D], bf16, kind="ExternalInput").ap()
    w1 = nc.dram_tensor("w1", [P, KT * F], bf16, kind="ExternalInput").ap()
    w2 = nc.dram_tensor("w2", [P, FT * D], bf16, kind="ExternalInput").ap()
    bo_v = nc.dram_tensor("bo_v", [P, KT], f32, kind="ExternalInput").ap()
    b1_v = nc.dram_tensor("b1_v", [P, FT], f32, kind="ExternalInput").ap()
    b2_v = nc.dram_tensor("b2_v", [P, KT], f32, kind="ExternalInput").ap()
    g1_v = nc.dram_tensor("g1_v", [P, KT], f32, kind="ExternalInput").ap()
    be1_v = nc.dram_tensor("be1_v", [P, KT], f32, kind="ExternalInput").ap()
    g2_v = nc.dram_tensor("g2_v", [P, KT], f32, kind="ExternalInput").ap()
    be2_v = nc.dram_tensor("be2_v", [P, KT], f32, kind="ExternalInput").ap()

    outT = nc.dram_tensor("outT", [D, TOK], f32, kind="ExternalOutput").ap()

    groups = [[0, 1, 2, 3], [4, 5, 6, 7]]

    with tile.TileContext(nc) as tc:
        sb = tc.alloc_tile_pool(name="sb", bufs=1)
        tr = tc.alloc_tile_pool(name="tr", bufs=3)
        ep = tc.alloc_tile_pool(name="ep", bufs=2)
        dram = tc.alloc_tile_pool(name="dram", bufs=1, space="DRAM")
        xp = tc.alloc_tile_pool(name="xp", bufs=1)

        # ---- constants / small inputs ----
        ones_f = sb.tile([P, 64], f32, name="ones_f", tag="ones_f")
        nc.vector.memset(ones_f[:], 1.0)
        ones_r = sb.tile([P, 1], bf16, name="ones_r", tag="ones_r")
        nc.vector.tensor_copy(ones_r[:], ones_f[:, 0:1])
        ones_b = sb.tile([P, 64], bf16, name="ones_b", tag="ones_b")
        nc.vector.tensor_copy(ones_b[:], ones_f[:])
        warm = sb.tile([1, 2], f32, name="warm", tag="warm")
        nc.scalar.activation(warm[:, 0:1], ones_f[0:1, 0:1], ACT.Exp)
        nc.scalar.activation(warm[:, 1:2], ones_f[0:1, 0:1], ACT.Ln)
        bo_t = sb.tile([P, KT], f32, name="bo_t", tag="bo_t")
        b1_t = sb.tile([P, FT], f32, name="b1_t", tag="b1_t")
        b2_t = sb.tile([P, KT], f32, name="b2_t", tag="b2_t")
        g1_t = sb.tile([P, KT], f32, name="g1_t", tag="g1_t")
        be1_t = sb.tile([P, KT], f32, name="be1_t", tag="be1_t")
        g2_t = sb.tile([P, KT], f32, name="g2_t", tag="g2_t")
        be2_t = sb.tile([P, KT], f32, name="be2_t", tag="be2_t")
        for t, src in [(bo_t, bo_v), (b1_t, b1_v), (b2_t, b2_v),
                       (g1_t, g1_v), (be1_t, be1_v), (g2_t, g2_v),
                       (be2_t, be2_v)]:
            nc.sync.dma_start(out=t[:], in_=src)

        # ---- load x^T shard: bf16 pre-tiled copy first (feeds LN1 and
        # the projections); the f32 copy (residual only) loads later, off
        # the startup critical path ----
        xbig = xp.tile([P, KT * TOK], f32, name="xbig", tag="xbig")
        x_tiles = [xbig[:, kt * TOK:(kt + 1) * TOK] for kt in range(KT)]

        # K^T and interleaved-V share one flat DRAM buffer -> ONE AllGather
        # (the collective's cost is fixed-overhead dominated).
        KN = D * TOK                    # K^T elements
        VN = TOK * KT * VBLK            # interleaved V elements
        KVN = KN + VN
        kv_sh = dram.tile([KVN], bf16, name="kv_sh", tag="kv_sh")
        kv_full = dram.tile([GRP * KVN], bf16, name="kv_full", tag="kv_full")
        wfp = tc.alloc_tile_pool(name="wfp", bufs=2)
        h2p = tc.alloc_tile_pool(name="h2p", bufs=1)

        for it in range(n_iters):
            # Pools in stack (release-LIFO) order: x2p > cp > wop > qp >
            # wqkv > hp > qkvps.  Weight DMAs issue first (prefetch).
            x2p = tc.alloc_tile_pool(name=f"x2p{it}", bufs=1)
            cp = tc.alloc_tile_pool(name=f"cp{it}", bufs=1)
            qp = tc.alloc_tile_pool(name=f"qp{it}", bufs=1)
            wqkv = tc.alloc_tile_pool(name=f"wqkv{it}", bufs=1)
            # wk/wv/wq/wo rotate through 2 slots (used strictly in sequence)
            wk_t = wqkv.tile([P, KT * D], bf16, name=f"wk_{it}", tag="wqkv",
                             bufs=2)
            wv_t = wqkv.tile([P, KT * D], bf16, name=f"wv_{it}", tag="wqkv",
                             bufs=2)
            wq_t = wqkv.tile([P, KT * D], bf16, name=f"wq_{it}", tag="wqkv",
                             bufs=2)
            wo_t = wqkv.tile([P, KT * D], bf16, name=f"wo_{it}", tag="wqkv",
                             bufs=2)
            nc.sync.dma_start(out=wk_t[:], in_=wk)
            nc.sync.dma_start(out=wv_t[:], in_=wv)
            nc.sync.dma_start(out=wq_t[:], in_=wq)
            w1g0 = wfp.tile([P, KT * 1024], bf16, name=f"w1g_{it}_0",
                            tag="w1g", bufs=2)
            w2g0 = wfp.tile([P, KT * D], bf16, name=f"w2g_{it}_0",
                            tag="w2g", bufs=1)

            # ================= LN1 =================
            xbfp = tc.alloc_tile_pool(name=f"xbfp{it}", bufs=1)
            xb_t = xbfp.tile([P, KT * TOK], bf16, name=f"xb_{it}", tag="xb")
            nc.sync.dma_start(out=xb_t[:], in_=xbf)
            xb_tiles = [xb_t[:, kt * TOK:(kt + 1) * TOK] for kt in range(KT)]
            hp = tc.alloc_tile_pool(name=f"hp{it}", bufs=1)
            h_tiles = _layer_norm(nc, tc, hp, ones_r, xb_tiles, g1_t, be1_t,
                                  f"ln1_{it}", pre_bf16=True)
            # bulk prefetch on the scalar (2nd HWDGE) queue, emitted after
            # LN1 so its row activations aren't stuck behind DMA dispatch
            nc.scalar.dma_start(out=wo_t[:], in_=wo)
            nc.scalar.dma_start(
                out=w1g0[:].rearrange("p (kt c) -> p kt c", c=1024),
                in_=w1.rearrange("p (kt f) -> p kt f", f=F)[:, :, 0:1024])
            nc.scalar.dma_start(out=w2g0[:], in_=w2[:, 0:KT * D])

            qkvps = tc.alloc_tile_pool(name=f"qkvps{it}", bufs=4, space="PSUM")

            # ---- K^T projection -> kT_sh -> AllGather ----
            kev = tr.tile([P, (TOK // P) * KT * VBLK], bf16, name=f"kev_{it}",
                          tag="kvev", bufs=1)
            for mt in range(KT):
                ps = qkvps.tile([P, TOK], f32, name=f"psk_{it}_{mt}", tag="qkv")
                for kt in range(KT):
                    nc.tensor.matmul(
                        ps[:], lhsT=wk_t[:, kt * D + mt * P:kt * D + (mt + 1) * P],
                        rhs=h_tiles[kt][:], start=(kt == 0), stop=(kt == KT - 1))
                nc.vector.tensor_copy(kev[:, mt * TOK:(mt + 1) * TOK], ps[:])
            nc.sync.dma_start(
                out=kv_sh[0:KN].rearrange("(kt p t) -> p kt t", p=P, t=TOK),
                in_=kev[:, 0:KT * TOK].rearrange("p (kt t) -> p kt t", t=TOK))

            # ---- V projection (interleaved [va|1|vb]) -> v_sh -> AllGather --
            vv = tr.tile([P, (TOK // P) * KT * VBLK], bf16, name=f"vv_{it}",
                         tag="kvev", bufs=1)
            # ones columns (cols 64 and 129 of each 130-block)
            nc.vector.tensor_copy(
                vv[:].rearrange("q (b c) -> q b c", c=DK1)[:, :, DK:DK + 1]
                .squeeze(2), ones_b[:])
            for mt in range(TOK // P):
                for nt in range(2):
                    ps = qkvps.tile([P, TOK], f32, name=f"psv_{it}_{mt}_{nt}",
                                    tag="qkv")
                    for kt in range(KT):
                        nc.tensor.matmul(
                            ps[:], lhsT=h_tiles[kt][:, mt * P:(mt + 1) * P],
                            rhs=wv_t[:, kt * D + nt * TOK:kt * D + (nt + 1) * TOK],
                            start=(kt == 0), stop=(kt == KT - 1))
                    # psum cols (pp, hh, dk) -> vv block mt, col (nt*4+pp)*129
                    base = mt * KT * VBLK + nt * 4 * VBLK
                    for hh in range(2):
                        nc.vector.tensor_copy(
                            vv[:, base:base + 4 * VBLK]
                            .rearrange("q (pp c) -> q pp c", c=VBLK)
                            [:, :, hh * DK1:hh * DK1 + DK],
                            ps[:].rearrange("q (pp hh dk) -> q pp hh dk",
                                            hh=2, dk=DK)[:, :, hh, :])
            nc.sync.dma_start(
                out=kv_sh[KN:KVN].rearrange("(mt q c) -> q mt c", q=P,
                                            c=KT * VBLK),
                in_=vv[:].rearrange("q (mt c) -> q mt c", c=KT * VBLK))
            nc.gpsimd.collective_compute(
                "AllGather", ALU.bypass, ins=[kv_sh[:].opt()],
                outs=[kv_full[:].opt()], replica_groups=groups)
            for kt in range(KT):
                nc.scalar.dma_start(out=xbig[:, kt * TOK:(kt + 1) * TOK],
                                    in_=xT[kt * P:(kt + 1) * P, :])

            # ---- Q^T projection (resident) ----
            qbig = qp.tile([P, KT * TOK], bf16, name=f"qbig_{it}", tag="qbig")
            for mt in range(KT):
                ps = qkvps.tile([P, TOK], f32, name=f"psq_{it}_{mt}", tag="qkv")
                for kt in range(KT):
                    nc.tensor.matmul(
                        ps[:], lhsT=wq_t[:, kt * D + mt * P:kt * D + (mt + 1) * P],
                        rhs=h_tiles[kt][:], start=(kt == 0), stop=(kt == KT - 1))
                nc.vector.tensor_copy(qbig[:, mt * TOK:(mt + 1) * TOK], ps[:])
            qkvps.release()
            hp.release()
            xbfp.release()

            # ================= attention =================
            scps = tc.alloc_tile_pool(name=f"scps{it}", bufs=2, space="PSUM")
            ctxps = tc.alloc_tile_pool(name=f"ctxps{it}", bufs=2, space="PSUM")
            cbig = cp.tile([P, KT * TOK], bf16, name=f"cbig_{it}", tag="cbig")
            for p in range(KT):
                ktp = tr.tile([P, S], bf16, name=f"ktp_{it}_{p}", tag="ktp",
                              bufs=2)
                nc.sync.dma_start(
                    out=ktp[:].rearrange("d (c t) -> d c t", c=GRP),
                    in_=kv_full[:].rearrange("(c x) -> c x", x=KVN)
                    [:, 0:KN].rearrange("c (d t) -> d c t", t=TOK)
                    [p * P:(p + 1) * P, :, :])
                vp = tr.tile([P, MT_S * VBLK], bf16, name=f"vp_{it}_{p}",
                             tag="vp", bufs=2)
                for c in range(GRP):
                    nc.sync.dma_start(
                        out=vp[:, c * 4 * VBLK:(c + 1) * 4 * VBLK]
                        .rearrange("q (m cc) -> q m cc", cc=VBLK),
                        in_=kv_full[c * KVN + KN:(c + 1) * KVN]
                        .rearrange("(m q cc) -> q m cc", q=P, cc=KT * VBLK)
                        [:, :, p * VBLK:(p + 1) * VBLK])

                ps0 = ctxps.tile([DK1, TOK], f32, name=f"ps0_{it}_{p}",
                                 tag="ps0")
                ps1 = ctxps.tile([DK1, TOK], f32, name=f"ps1_{it}_{p}",
                                 tag="ps1")
                q_p = qbig[:, p * TOK:(p + 1) * TOK]
                ps_sc = None
                e_t = None
                nb = 0
                for i in range(2 * MT_S):
                    mt, half = i >> 1, i & 1
                    j = i % NBLK
                    if j == 0:
                        nb = min(NBLK, 2 * MT_S - i)
                        ps_sc = scps.tile([P, NBLK * TOK], f32,
                                          name=f"pssc_{it}_{p}_{i}", tag="pssc")
                        e_t = ep.tile([P, NBLK * TOK], bf16,
                                      name=f"e_{it}_{p}_{i}", tag="e")
                    nc.tensor.matmul(
                        ps_sc[:, j * TOK:(j + 1) * TOK],
                        lhsT=ktp[half * DK:(half + 1) * DK, mt * P:(mt + 1) * P],
                        rhs=q_p[half * DK:(half + 1) * DK, :],
                        start=True, stop=True)
                    if j == nb - 1:
                        nc.scalar.activation(e_t[:, 0:nb * TOK],
                                             ps_sc[:, 0:nb * TOK],
                                             ACT.Exp, scale=SCALE)
                        for jj in range(nb):
                            ii = i - nb + 1 + jj
                            mtt, hf = ii >> 1, ii & 1
                            out_ap = ps0[:] if hf == 0 else ps1[:]
                            lhsT = vp[:, mtt * VBLK + hf * DK1:
                                      mtt * VBLK + (hf + 1) * DK1]
                            nc.tensor.matmul(
                                out_ap, lhsT=lhsT,
                                rhs=e_t[:, jj * TOK:(jj + 1) * TOK],
                                start=(mtt == 0), stop=(mtt == MT_S - 1))
                # normalize: ctx = ctx_unnorm / sumexp
                ctx_p = cbig[:, p * TOK:(p + 1) * TOK]
                for hf, psx in ((0, ps0), (1, ps1)):
                    rec = tr.tile([1, TOK], bf16, name=f"rec{hf}_{it}_{p}",
                                  tag="rec", bufs=2)
                    with nc.allow_low_precision(
                            reason="softmax 1/sumexp in bf16 is plenty"):
                        nc.vector.reciprocal(rec[:], psx[DK:DK1, :])
                    rbc = tr.tile([DK, TOK], bf16, name=f"rbc{hf}_{it}_{p}",
                                  tag="recbc", bufs=2)
                    nc.gpsimd.partition_broadcast(rbc[:], rec[:])
                    if hf == 0:
                        nc.vector.tensor_mul(ctx_p[0:DK, :], psx[0:DK, :],
                                             rbc[:])
                    else:
                        sh = tr.tile([DK, TOK], bf16, name=f"sh_{it}_{p}",
                                     tag="shift", bufs=2)
                        nc.vector.tensor_mul(sh[:], psx[0:DK, :], rbc[:])
                        nc.sync.dma_start(out=ctx_p[DK:P, :], in_=sh[:])
            ctxps.release()
            scps.release()

            # ======== W_o + residual -> x2, LN2 stats interleaved ========
            ln2_st = _ln_begin(nc, tc, f"ln2_{it}")
            wops = tc.alloc_tile_pool(name=f"wops{it}", bufs=4, space="PSUM")
            x2big = x2p.tile([P, KT * TOK], f32, name=f"x2big_{it}", tag="x2")
            x2_tiles = [x2big[:, mt * TOK:(mt + 1) * TOK] for mt in range(KT)]
            for mt in range(KT):
                ps = wops.tile([P, TOK], f32, name=f"pso_{it}_{mt}", tag="wo")
                for kt in range(KT):
                    nc.tensor.matmul(
                        ps[:], lhsT=wo_t[:, kt * D + mt * P:kt * D + (mt + 1) * P],
                        rhs=cbig[:, kt * TOK:(kt + 1) * TOK],
                        start=(kt == 0), stop=(kt == KT - 1))
                nc.vector.scalar_tensor_tensor(x2_tiles[mt], ps[:],
                                               bo_t[:, mt:mt + 1],
                                               x_tiles[mt], ALU.add, ALU.add)
                _ln_accum(nc, ln2_st, ones_r, x2_tiles[mt], mt)
            wops.release()
            h2_tiles = _ln_finish(nc, ln2_st, h2p, x2_tiles, g2_t, be2_t)
            wqkv.release()
            qp.release()

            # ================= FFN =================
            apool = tc.alloc_tile_pool(name=f"ap{it}", bufs=1)
            f1ps = tc.alloc_tile_pool(name=f"f1ps{it}", bufs=4, space="PSUM")
            f2ps = tc.alloc_tile_pool(name=f"f2ps{it}", bufs=4, space="PSUM")
            for mg in range(4):
                if mg == 0:
                    w1g, w2g = w1g0, w2g0
                else:
                    w1g = wfp.tile([P, KT * 1024], bf16,
                                   name=f"w1g_{it}_{mg}", tag="w1g", bufs=2)
                    nc.scalar.dma_start(
                        out=w1g[:].rearrange("p (kt c) -> p kt c", c=1024),
                        in_=w1.rearrange("p (kt f) -> p kt f", f=F)
                        [:, :, mg * 1024:(mg + 1) * 1024])
                    w2g = wfp.tile([P, KT * D], bf16, name=f"w2g_{it}_{mg}",
                                   tag="w2g", bufs=1)
                    nc.scalar.dma_start(
                        out=w2g[:], in_=w2[:, mg * KT * D:(mg + 1) * KT * D])
                abig = apool.tile([P, KT * TOK], bf16, name=f"a_{it}_{mg}",
                                  tag="a", bufs=2)
                for mt in range(KT):
                    m = mg * KT + mt
                    ps = f1ps.tile([P, TOK], f32, name=f"psf1_{it}_{m}",
                                   tag="f1")
                    for kt in range(KT):
                        nc.tensor.matmul(
                            ps[:],
                            lhsT=w1g[:, kt * 1024 + mt * P:kt * 1024 + (mt + 1) * P],
                            rhs=h2_tiles[kt][:], start=(kt == 0),
                            stop=(kt == KT - 1))
                    nc.vector.tensor_scalar(abig[:, mt * TOK:(mt + 1) * TOK],
                                            ps[:], b1_t[:, m:m + 1], 0.0,
                                            ALU.add, ALU.max)
                for mt in range(KT):
                    ps2 = f2ps.tile([P, TOK], f32, name=f"psf2_{it}_{mg}_{mt}",
                                    tag="f2")
                    for kt in range(KT):
                        nc.tensor.matmul(
                            ps2[:],
                            lhsT=w2g[:, kt * D + mt * P:kt * D + (mt + 1) * P],
                            rhs=abig[:, kt * TOK:(kt + 1) * TOK],
                            start=(kt == 0), stop=(kt == KT - 1))
                    nc.vector.tensor_add(x2_tiles[mt], x2_tiles[mt], ps2[:])
            f2ps.release()
            f1ps.release()
            apool.release()
            cp.release()

            # ---- final bias + store ----
            for mt in range(KT):
                ot = tr.tile([P, TOK], f32, name=f"ot_{it}_{mt}", tag="ot",
                             bufs=2)
                nc.vector.tensor_scalar_add(ot[:], x2_tiles[mt],
                                            b2_t[:, mt:mt + 1])
                nc.sync.dma_start(out=outT[mt * P:(mt + 1) * P, :], in_=ot[:])

            x2p.release()

        h2p.release()
        wfp.release()
        xp.release()
        dram.release()
        ep.release()
        tr.release()
        sb.release()

    nc.compile()
    return nc


_NC_CACHE = {}


def _get_nc(n_iters: int = 1):
    if n_iters not in _NC_CACHE:
        _NC_CACHE[n_iters] = build(n_iters)
    return _NC_CACHE[n_iters]


def _pack_vec(v: np.ndarray, nt: int) -> np.ndarray:
    return np.ascontiguousarray(v.reshape(nt, P).T, dtype=np.float32)


def make_in_maps(inputs: dict) -> list:
    x = np.asarray(inputs["x"], dtype=np.float32)
    B, S_, D_ = x.shape
    assert (B, S_, D_) == (2, S, D)
    xf = x.reshape(B * S_, D_)

    shared = {
        "wq": tile_lhsT(inputs["W_q"]),
        "wk": tile_lhsT(inputs["W_k"]),
        "wv": tile_lhsT(inputs["W_v"]),
        "wo": tile_lhsT(inputs["W_o"]),
        "w1": tile_lhsT(inputs["W1"]),
        "w2": tile_lhsT(inputs["W2"]),
        "bo_v": _pack_vec(np.asarray(inputs["b_o"], np.float32), KT),
        "b1_v": _pack_vec(np.asarray(inputs["b1"], np.float32), FT),
        "b2_v": _pack_vec(np.asarray(inputs["b2"], np.float32), KT),
        "g1_v": _pack_vec(np.asarray(inputs["ln1_g"], np.float32), KT),
        "be1_v": _pack_vec(np.asarray(inputs["ln1_b"], np.float32), KT),
        "g2_v": _pack_vec(np.asarray(inputs["ln2_g"], np.float32), KT),
        "be2_v": _pack_vec(np.asarray(inputs["ln2_b"], np.float32), KT),
    }
    in_maps = []
    for c in range(N_CORES):
        xT_c = np.ascontiguousarray(xf[c * TOK:(c + 1) * TOK, :].T)
        in_maps.append({"xT": xT_c, "xbf": tile_lhsT(xT_c), **shared})
    return in_maps


def run(inputs: dict, trace: bool = False):
    in_maps = make_in_maps(inputs)
    nc = _get_nc(1)
    res = run_bass_kernel_spmd(nc, in_maps, list(range(N_CORES)), trace=trace)
    out = np.empty((2 * S, D), dtype=np.float32)
    for c in range(N_CORES):
        out[c * TOK:(c + 1) * TOK, :] = res.results[c]["outT"].T
    return out.reshape(2, S, D), res


def kernel(**inputs) -> np.ndarray:
    out, _ = run(inputs, trace=False)
    return out
